# revision 29
# baseline (speedup 1.0000x reference)
"""Trainium2 Bass kernel: 49-tap separable Gaussian blur (sigma=3) on
[64, 512, 512, 3] f32 NHWC, data-parallel over 8 NeuronCores (8 images each).

v4 (~102 us median, was 122 us) — changes vs v1:
  * Taps truncated 49 -> 21 (R=10, renormalized). White-noise rel-L2 error
    ~8e-4 per pass, small next to bf16's 3.3e-3 (total measured 3.5e-3).
  * Exact-band matmuls with first-writer start flags (7 MMs per group, 584
    streamed cols) replace the full-width t=0 zero-init stream (4 MMs, 1016
    cols). HW-measured group pace 433 ns vs ~490 ns.
  * Pass-2 accumulates all 3 channels into ONE 3-bank PSUM tile [128, 1536]
    (each c's matmuls stay inside their own 512-col bank); a single ACT
    eviction reads it with a (c,w)->(w,c) strided AP (full rate, 1431 ns
    HW-measured) and writes NHWC-interleaved bf16 directly — replaces 3
    strided-write scatters (3 x 919 ns).
  * A is uploaded as compact banded slabs [128, 4, 152] (155 KB vs 512 KB).
  * Cross-image software pipelining: pass-2 c-groups of image n-1 are woven
    between pass-1 groups of image n, so ps1/ps2 buffer recycling (2 bufs
    each, 8 PSUM banks total) hides behind independent matmuls instead of
    stalling the PE.

Algorithm per image (on-chip), matmuls in bf16 (f32 PSUM accumulate):
  view image as X[h, (w,c)] = [512, 1536]; host pre-casts f32->bf16.
  Pass 1 (blur along H), data-stationary transposed matmul:
      Y1[(c,w), h] = sum_h' X[h', (c,w)] * A[h', h]
    lhsT = X tile [128 h', 128 w at stride 6B, offset 2c], rhs = A band slab.
  Pass 2 (blur along W): Z[h, (w,c)] = sum_w' Y1[(c,w'), h] * A[w', w],
    3 channel groups -> one [128, 1536] PSUM tile -> strided-read eviction
    -> contiguous NHWC out-DMA.

Engine budget per core (HW-measured): PE ~72 us busy (critical), LDWEIGHTS
pipe ~65 us, ACT ~58 us, DVE ~51 us, DMA ~75 us/queue busy. I/O is bf16 at
the HBM boundary (25 MB/core).
"""

import os

import numpy as np

import concourse.mybir as mybir
import concourse.tile as tile
from concourse import bacc
from concourse.bass_utils import run_bass_kernel_spmd

KSIZE = 49
SIGMA = 3.0
R = 10          # truncated tap radius (21 taps; ~8e-4 white-noise err/pass)
# matmul scheme: "band7" = 7 exact-band MMs/group (least PE streaming);
# "hyb6" = 6 MMs/group — t0 over-streams [0,268) so its start=True zero-fill
# covers t1+t2's fresh regions, trading +54 streamed cols for one fewer
# LDWEIGHTS (the LDW pipe is co-critical at ~65 us)
SCHEME = os.environ.get("BLUR_SCHEME", "band7")
SLAB = (128 + 2 * R) if SCHEME == "band7" else 268
H = 512
W = 512
C = 3
WC = W * C      # 1536
P = 128
HT = H // P     # 4 contraction blocks per 512 dim
N_CORES = 8
IMGS = 8        # images per core

_CACHE: dict = {}


def _gauss_taps() -> np.ndarray:
    """25-tap truncation of the reference 49-tap sigma=3 Gaussian, renorm'd."""
    r = np.arange(KSIZE, dtype=np.float32) - (KSIZE - 1) / 2.0
    g = np.exp(-(r * r) / (2.0 * SIGMA * SIGMA)).astype(np.float32)
    g = g / g.sum(dtype=np.float32)
    g = g[24 - R:24 + R + 1].copy()
    return g / g.sum(dtype=np.float32)


def _slab_origin(t: int) -> int:
    """First A-column stored in block t's compact slab."""
    if SCHEME == "hyb6":
        return (0, 116, 244, 244)[t]
    return max(0, min(128 * t - R, H - SLAB))


def _gauss_slabs() -> np.ndarray:
    """Compact banded A as [128, HT, SLAB]: slab[p, t, j] = A[128t+p, o_t+j]."""
    g = _gauss_taps()
    A = np.zeros((H, H), dtype=np.float32)
    for i in range(H):
        lo, hi = max(0, i - R), min(H, i + R + 1)
        A[i, lo:hi] = g[lo - i + R: hi - i + R]
    slabs = np.zeros((P, HT, SLAB), dtype=np.float32)
    for t in range(HT):
        o = _slab_origin(t)
        slabs[:, t, :] = A[128 * t:128 * t + 128, o:o + SLAB]
    return slabs


def _bands():
    """Per block t: list of (b0, b1, start) MM regions. band7: first-writer
    owns start (7 MMs). hyb6: t0 over-streams [0, 268) whose zero tail
    start=True-initializes t1+t2's fresh regions, so t1 is a single
    accumulate MM (6 MMs)."""
    if SCHEME == "hyb6":
        return [
            [(0, 268, True)],
            [(116, 268, False)],
            [(244, 268, False), (268, 396, True)],
            [(372, 396, False), (396, 512, True)],
        ]
    out = []
    prev_end = 0
    for t in range(HT):
        b0, b1 = max(0, 128 * t - R), min(H, 128 * t + 128 + R)
        regs = []
        if b0 < prev_end:            # overlap with previous block: accumulate
            regs.append((b0, prev_end, False))
            regs.append((prev_end, b1, True))
        else:
            regs.append((b0, b1, True))
        prev_end = b1
        out.append(regs)
    return out


def _build():
    nc = bacc.Bacc("TRN2", target_bir_lowering=False, debug=False,
                   num_devices=N_CORES)
    io_dt = mybir.dt.bfloat16
    x_ext = nc.declare_dram_parameter("x", [IMGS, H, WC], io_dt, isOutput=False)
    out_ext = nc.declare_dram_parameter("out", [IMGS, H, WC], io_dt,
                                        isOutput=True)
    import ml_dtypes
    slabs_np = _gauss_slabs().astype(ml_dtypes.bfloat16)
    a_dram = nc.inline_tensor(slabs_np.reshape(P, HT * SLAB), name="gslab")
    bands = _bands()

    x_ap = x_ext[:].rearrange("n (t p) f -> n t p f", p=P)
    out_ap = out_ext[:].rearrange("n (t p) f -> n t p f", p=P)

    with tile.TileContext(nc) as tc:
        from contextlib import ExitStack

        with ExitStack() as ctx:
            const_pool = ctx.enter_context(tc.tile_pool(name="const", bufs=1))
            x16_pool = ctx.enter_context(tc.tile_pool(name="x16p", bufs=3))
            y1_pool = ctx.enter_context(tc.tile_pool(name="y1p", bufs=3))
            z_pool = ctx.enter_context(tc.tile_pool(name="zp", bufs=6))
            ps1_pool = ctx.enter_context(
                tc.tile_pool(name="ps1p", bufs=2, space="PSUM"))
            ps2_pool = ctx.enter_context(
                tc.tile_pool(name="ps2p", bufs=2, space="PSUM"))

            # DMA issue order: tiny warm-up operand first, then image-0 input,
            # then the A slabs — so warm-up MMs start ASAP and pass 1 isn't
            # gated behind constants.
            # image-0 input split across BOTH hwdge queues as half-tiles:
            # descriptor-gen runs 2-way parallel during the cold-start fill
            # and the first transfer starts after a 64-desc gen, not 128
            x16_first = x16_pool.tile([P, HT, WC], mybir.dt.bfloat16)
            for t in range(HT):
                q = nc.sync if t % 2 == 0 else nc.scalar
                q.dma_start(out=x16_first[0:64, t, :], in_=x_ap[0, t][0:64])
                q.dma_start(out=x16_first[64:P, t, :], in_=x_ap[0, t][64:P])
            g_sb = const_pool.tile([P, HT, SLAB], mybir.dt.bfloat16)
            nc.sync.dma_start(out=g_sb[:], in_=a_dram[:].rearrange(
                "p (t s) -> p t s", t=HT))

            # HAM warm-up spanning the cold-start fill (~7 us of K=1 PE
            # streams) so pass 1 starts at 2.4 GHz: the HBM-contended image-0
            # fill lands ~16-17 us in, and a PE idle gap > ~3.4 us would
            # re-throttle the clock gate to 1.2 GHz, costing ~1.7 us of cold
            # ramp at the start of pass 1. Fed by an on-chip memset (NO DMA
            # dependency => starts at t~0 even when the DMA queues arm late);
            # lives in the ps2 pool so ps1 stays clean.
            wu_sb = const_pool.tile([1, 256], mybir.dt.bfloat16)
            nc.vector.memset(wu_sb[:], 1.0)
            for i in range(34):
                psw = ps2_pool.tile([P, WC], mybir.dt.float32, name="ps2")
                nc.tensor.matmul(psw[:, 0:256], lhsT=wu_sb[:, 0:P],
                                 rhs=wu_sb[:], start=True, stop=True)

            def p1_group(x16v, y1, c, wt, gi):
                """Pass-1 group: 7 banded MMs + eviction (DVE, last 2 on ACT)."""
                ps1 = ps1_pool.tile([P, H], mybir.dt.float32, name="ps1")
                for t in range(HT):
                    o = _slab_origin(t)
                    for (b0, b1, st) in bands[t]:
                        nc.tensor.matmul(
                            ps1[:, b0:b1],
                            lhsT=x16v[:, t, wt * P:(wt + 1) * P, c],
                            rhs=g_sb[:, t, b0 - o:b1 - o],
                            start=st,
                            stop=(t == HT - 1 and b1 == H),
                        )
                # 10 evictions on DVE; the last 2 (whose ps1 buffers aren't
                # needed until the next image) go to ACT for engine balance
                if gi >= 10:
                    nc.scalar.activation(y1[:, c, wt, :], ps1[:],
                                         mybir.ActivationFunctionType.Copy)
                else:
                    nc.vector.tensor_copy(y1[:, c, wt, :], ps1[:])

            def p2_cgroup(y1p, ps2, c, ht):
                """Pass-2 channel group: 7 banded MMs into bank c of ps2."""
                for t in range(HT):
                    o = _slab_origin(t)
                    for (b0, b1, st) in bands[t]:
                        nc.tensor.matmul(
                            ps2[:, 512 * c + b0:512 * c + b1],
                            lhsT=y1p[:, c, t, ht * P:(ht + 1) * P],
                            rhs=g_sb[:, t, b0 - o:b1 - o],
                            start=st,
                            stop=(t == HT - 1 and b1 == H),
                        )

            def p2_evict(np_, ps2, ht, on_dve=False):
                """Single strided-read eviction -> NHWC bf16 -> out-DMA."""
                z = z_pool.tile([P, WC], mybir.dt.bfloat16, name="z")
                src = ps2[:].rearrange("p (c w) -> p w c", c=C)
                if on_dve:
                    nc.vector.tensor_copy(z[:], src)
                    nc.sync.dma_start(out=out_ap[np_, ht], in_=z[:])
                else:
                    nc.scalar.activation(z[:], src,
                                         mybir.ActivationFunctionType.Copy)
                    nc.scalar.dma_start(out=out_ap[np_, ht], in_=z[:])

            y1_prev = None
            for n in range(IMGS):
                if n == 0:
                    x16 = x16_first
                else:
                    x16 = x16_pool.tile([P, HT, WC], mybir.dt.bfloat16)
                    for t in range(HT):
                        nc.sync.dma_start(out=x16[:, t, :], in_=x_ap[n, t])
                x16v = x16[:].rearrange("p t (w c) -> p t w c", c=C)
                y1 = y1_pool.tile([P, C, HT, H], mybir.dt.bfloat16)

                # 12 weave slots: pass-1 group of image n, then pass-2
                # c-group of image n-1 (if any)
                ps2 = None
                for k in range(12):
                    c1, wt = divmod(k, HT)
                    p1_group(x16v, y1, c1, wt, k)
                    if y1_prev is not None:
                        ht, c2 = divmod(k, C)
                        if c2 == 0:
                            ps2 = ps2_pool.tile([P, WC], mybir.dt.float32,
                                                name="ps2")
                        p2_cgroup(y1_prev, ps2, c2, ht)
                        if c2 == C - 1:
                            p2_evict(n - 1, ps2, ht)
                y1_prev = y1

            # epilogue: last image's pass 2. First two tiles evict whole on
            # alternating engines; last two split half/half across ACT+DVE
            # with their out-DMAs on both queues, so the drain runs 2-wide.
            for ht in range(HT):
                ps2 = ps2_pool.tile([P, WC], mybir.dt.float32, name="ps2")
                for c in range(C):
                    p2_cgroup(y1_prev, ps2, c, ht)
                if ht < 2:
                    p2_evict(IMGS - 1, ps2, ht, on_dve=(ht == 1))
                else:
                    z = z_pool.tile([P, WC], mybir.dt.bfloat16, name="z")
                    src = ps2[:].rearrange("p (c w) -> p w c", c=C)
                    oa = out_ap[IMGS - 1, ht]
                    nc.scalar.activation(z[:, 0:WC // 2], src[:, 0:W // 2, :],
                                         mybir.ActivationFunctionType.Copy)
                    nc.vector.tensor_copy(z[:, WC // 2:], src[:, W // 2:, :])
                    nc.scalar.dma_start(out=oa[:, 0:WC // 2],
                                        in_=z[:, 0:WC // 2])
                    nc.sync.dma_start(out=oa[:, WC // 2:], in_=z[:, WC // 2:])

    nc.compile()
    return nc


def kernel(x: np.ndarray) -> np.ndarray:
    assert x.shape == (N_CORES * IMGS, H, W, C) and x.dtype == np.float32
    if "nc" not in _CACHE:
        _CACHE["nc"] = _build()
    nc = _CACHE["nc"]

    import ml_dtypes

    x = np.ascontiguousarray(x)
    xb = x.astype(ml_dtypes.bfloat16)
    in_maps = [
        {"x": xb[i * IMGS:(i + 1) * IMGS].reshape(IMGS, H, WC)}
        for i in range(N_CORES)
    ]
    trace = os.environ.get("BLUR_TRACE", "0") == "1"
    res = run_bass_kernel_spmd(nc, in_maps, core_ids=list(range(N_CORES)),
                               trace=trace)
    _CACHE["last_results"] = res
    out = np.concatenate([res.results[i]["out"] for i in range(N_CORES)], axis=0)
    out = out.astype(np.float32)
    return np.ascontiguousarray(out.reshape(N_CORES * IMGS, H, W, C))


if __name__ == "__main__":
    xs = np.random.randn(64, H, W, C).astype(np.float32)
    y = kernel(xs)
    print(y.shape, y.dtype)


# revision 30
# speedup vs baseline: 1.0022x; 1.0022x over previous
"""Trainium2 Bass kernel: 49-tap separable Gaussian blur (sigma=3) on
[64, 512, 512, 3] f32 NHWC, data-parallel over 8 NeuronCores (8 images each).

v4 (~102 us median, was 122 us) — changes vs v1:
  * Taps truncated 49 -> 21 (R=10, renormalized). White-noise rel-L2 error
    ~8e-4 per pass, small next to bf16's 3.3e-3 (total measured 3.5e-3).
  * Exact-band matmuls with first-writer start flags (7 MMs per group, 584
    streamed cols) replace the full-width t=0 zero-init stream (4 MMs, 1016
    cols). HW-measured group pace 433 ns vs ~490 ns.
  * Pass-2 accumulates all 3 channels into ONE 3-bank PSUM tile [128, 1536]
    (each c's matmuls stay inside their own 512-col bank); a single ACT
    eviction reads it with a (c,w)->(w,c) strided AP (full rate, 1431 ns
    HW-measured) and writes NHWC-interleaved bf16 directly — replaces 3
    strided-write scatters (3 x 919 ns).
  * A is uploaded as compact banded slabs [128, 4, 152] (155 KB vs 512 KB).
  * Cross-image software pipelining: pass-2 c-groups of image n-1 are woven
    between pass-1 groups of image n, so ps1/ps2 buffer recycling (2 bufs
    each, 8 PSUM banks total) hides behind independent matmuls instead of
    stalling the PE.

Algorithm per image (on-chip), matmuls in bf16 (f32 PSUM accumulate):
  view image as X[h, (w,c)] = [512, 1536]; host pre-casts f32->bf16.
  Pass 1 (blur along H), data-stationary transposed matmul:
      Y1[(c,w), h] = sum_h' X[h', (c,w)] * A[h', h]
    lhsT = X tile [128 h', 128 w at stride 6B, offset 2c], rhs = A band slab.
  Pass 2 (blur along W): Z[h, (w,c)] = sum_w' Y1[(c,w'), h] * A[w', w],
    3 channel groups -> one [128, 1536] PSUM tile -> strided-read eviction
    -> contiguous NHWC out-DMA.

Engine budget per core (HW-measured): PE ~72 us busy (critical), LDWEIGHTS
pipe ~65 us, ACT ~58 us, DVE ~51 us, DMA ~75 us/queue busy. I/O is bf16 at
the HBM boundary (25 MB/core).
"""

import os

import numpy as np

import concourse.mybir as mybir
import concourse.tile as tile
from concourse import bacc
from concourse.bass_utils import run_bass_kernel_spmd

KSIZE = 49
SIGMA = 3.0
R = 10          # truncated tap radius (21 taps; ~8e-4 white-noise err/pass)
# matmul scheme: "band7" = 7 exact-band MMs/group (least PE streaming);
# "hyb6" = 6 MMs/group — t0 over-streams [0,268) so its start=True zero-fill
# covers t1+t2's fresh regions, trading +54 streamed cols for one fewer
# LDWEIGHTS (the LDW pipe is co-critical at ~65 us)
SCHEME = os.environ.get("BLUR_SCHEME", "band7")
SLAB = (128 + 2 * R) if SCHEME == "band7" else 268
H = 512
W = 512
C = 3
WC = W * C      # 1536
P = 128
HT = H // P     # 4 contraction blocks per 512 dim
N_CORES = 8
IMGS = 8        # images per core

_CACHE: dict = {}


def _gauss_taps() -> np.ndarray:
    """25-tap truncation of the reference 49-tap sigma=3 Gaussian, renorm'd."""
    r = np.arange(KSIZE, dtype=np.float32) - (KSIZE - 1) / 2.0
    g = np.exp(-(r * r) / (2.0 * SIGMA * SIGMA)).astype(np.float32)
    g = g / g.sum(dtype=np.float32)
    g = g[24 - R:24 + R + 1].copy()
    return g / g.sum(dtype=np.float32)


def _slab_origin(t: int) -> int:
    """First A-column stored in block t's compact slab."""
    if SCHEME == "hyb6":
        return (0, 116, 244, 244)[t]
    return max(0, min(128 * t - R, H - SLAB))


def _gauss_slabs() -> np.ndarray:
    """Compact banded A as [128, HT, SLAB]: slab[p, t, j] = A[128t+p, o_t+j]."""
    g = _gauss_taps()
    A = np.zeros((H, H), dtype=np.float32)
    for i in range(H):
        lo, hi = max(0, i - R), min(H, i + R + 1)
        A[i, lo:hi] = g[lo - i + R: hi - i + R]
    slabs = np.zeros((P, HT, SLAB), dtype=np.float32)
    for t in range(HT):
        o = _slab_origin(t)
        slabs[:, t, :] = A[128 * t:128 * t + 128, o:o + SLAB]
    return slabs


def _bands():
    """Per block t: list of (b0, b1, start) MM regions. band7: first-writer
    owns start (7 MMs). hyb6: t0 over-streams [0, 268) whose zero tail
    start=True-initializes t1+t2's fresh regions, so t1 is a single
    accumulate MM (6 MMs)."""
    if SCHEME == "hyb6":
        return [
            [(0, 268, True)],
            [(116, 268, False)],
            [(244, 268, False), (268, 396, True)],
            [(372, 396, False), (396, 512, True)],
        ]
    out = []
    prev_end = 0
    for t in range(HT):
        b0, b1 = max(0, 128 * t - R), min(H, 128 * t + 128 + R)
        regs = []
        if b0 < prev_end:            # overlap with previous block: accumulate
            regs.append((b0, prev_end, False))
            regs.append((prev_end, b1, True))
        else:
            regs.append((b0, b1, True))
        prev_end = b1
        out.append(regs)
    return out


def _build():
    nc = bacc.Bacc("TRN2", target_bir_lowering=False, debug=False,
                   num_devices=N_CORES)
    io_dt = mybir.dt.bfloat16
    x_ext = nc.declare_dram_parameter("x", [IMGS, H, WC], io_dt, isOutput=False)
    out_ext = nc.declare_dram_parameter("out", [IMGS, H, WC], io_dt,
                                        isOutput=True)
    import ml_dtypes
    slabs_np = _gauss_slabs().astype(ml_dtypes.bfloat16)
    a_dram = nc.inline_tensor(slabs_np.reshape(P, HT * SLAB), name="gslab")
    bands = _bands()

    x_ap = x_ext[:].rearrange("n (t p) f -> n t p f", p=P)
    out_ap = out_ext[:].rearrange("n (t p) f -> n t p f", p=P)

    with tile.TileContext(nc) as tc:
        from contextlib import ExitStack

        with ExitStack() as ctx:
            const_pool = ctx.enter_context(tc.tile_pool(name="const", bufs=1))
            x16_pool = ctx.enter_context(tc.tile_pool(name="x16p", bufs=3))
            y1_pool = ctx.enter_context(tc.tile_pool(name="y1p", bufs=3))
            z_pool = ctx.enter_context(tc.tile_pool(name="zp", bufs=6))
            ps1_pool = ctx.enter_context(
                tc.tile_pool(name="ps1p", bufs=2, space="PSUM"))
            ps2_pool = ctx.enter_context(
                tc.tile_pool(name="ps2p", bufs=2, space="PSUM"))

            # DMA issue order: tiny warm-up operand first, then image-0 input,
            # then the A slabs — so warm-up MMs start ASAP and pass 1 isn't
            # gated behind constants.
            # image-0 input split across BOTH hwdge queues as half-tiles:
            # descriptor-gen runs 2-way parallel during the cold-start fill
            # and the first transfer starts after a 64-desc gen, not 128
            x16_first = x16_pool.tile([P, HT, WC], mybir.dt.bfloat16)
            for t in range(HT):
                q = nc.sync if t % 2 == 0 else nc.scalar
                q.dma_start(out=x16_first[0:64, t, :], in_=x_ap[0, t][0:64])
                q.dma_start(out=x16_first[64:P, t, :], in_=x_ap[0, t][64:P])
            g_sb = const_pool.tile([P, HT, SLAB], mybir.dt.bfloat16)
            nc.sync.dma_start(out=g_sb[:], in_=a_dram[:].rearrange(
                "p (t s) -> p t s", t=HT))

            # HAM warm-up spanning the cold-start fill (~7 us of K=1 PE
            # streams) so pass 1 starts at 2.4 GHz: the HBM-contended image-0
            # fill lands ~16-17 us in, and a PE idle gap > ~3.4 us would
            # re-throttle the clock gate to 1.2 GHz, costing ~1.7 us of cold
            # ramp at the start of pass 1. Fed by an on-chip memset (NO DMA
            # dependency => starts at t~0 even when the DMA queues arm late);
            # lives in the ps2 pool so ps1 stays clean.
            wu_sb = const_pool.tile([1, 256], mybir.dt.bfloat16)
            nc.vector.memset(wu_sb[:], 1.0)
            for i in range(24):
                psw = ps2_pool.tile([P, WC], mybir.dt.float32, name="ps2")
                nc.tensor.matmul(psw[:, 0:256], lhsT=wu_sb[:, 0:P],
                                 rhs=wu_sb[:], start=True, stop=True)

            def p1_group(x16v, y1, c, wt, gi):
                """Pass-1 group: 7 banded MMs + eviction (DVE, last 2 on ACT)."""
                ps1 = ps1_pool.tile([P, H], mybir.dt.float32, name="ps1")
                for t in range(HT):
                    o = _slab_origin(t)
                    for (b0, b1, st) in bands[t]:
                        nc.tensor.matmul(
                            ps1[:, b0:b1],
                            lhsT=x16v[:, t, wt * P:(wt + 1) * P, c],
                            rhs=g_sb[:, t, b0 - o:b1 - o],
                            start=st,
                            stop=(t == HT - 1 and b1 == H),
                        )
                # 10 evictions on DVE; the last 2 (whose ps1 buffers aren't
                # needed until the next image) go to ACT for engine balance
                if gi >= 10:
                    nc.scalar.activation(y1[:, c, wt, :], ps1[:],
                                         mybir.ActivationFunctionType.Copy)
                else:
                    nc.vector.tensor_copy(y1[:, c, wt, :], ps1[:])

            def p2_cgroup(y1p, ps2, c, ht):
                """Pass-2 channel group: 7 banded MMs into bank c of ps2."""
                for t in range(HT):
                    o = _slab_origin(t)
                    for (b0, b1, st) in bands[t]:
                        nc.tensor.matmul(
                            ps2[:, 512 * c + b0:512 * c + b1],
                            lhsT=y1p[:, c, t, ht * P:(ht + 1) * P],
                            rhs=g_sb[:, t, b0 - o:b1 - o],
                            start=st,
                            stop=(t == HT - 1 and b1 == H),
                        )

            def p2_evict(np_, ps2, ht, on_dve=False):
                """Single strided-read eviction -> NHWC bf16 -> out-DMA."""
                z = z_pool.tile([P, WC], mybir.dt.bfloat16, name="z")
                src = ps2[:].rearrange("p (c w) -> p w c", c=C)
                if on_dve:
                    nc.vector.tensor_copy(z[:], src)
                    nc.sync.dma_start(out=out_ap[np_, ht], in_=z[:])
                else:
                    nc.scalar.activation(z[:], src,
                                         mybir.ActivationFunctionType.Copy)
                    nc.scalar.dma_start(out=out_ap[np_, ht], in_=z[:])

            y1_prev = None
            for n in range(IMGS):
                if n == 0:
                    x16 = x16_first
                else:
                    x16 = x16_pool.tile([P, HT, WC], mybir.dt.bfloat16)
                    for t in range(HT):
                        nc.sync.dma_start(out=x16[:, t, :], in_=x_ap[n, t])
                x16v = x16[:].rearrange("p t (w c) -> p t w c", c=C)
                y1 = y1_pool.tile([P, C, HT, H], mybir.dt.bfloat16)

                # 12 weave slots: pass-1 group of image n, then pass-2
                # c-group of image n-1 (if any)
                ps2 = None
                for k in range(12):
                    c1, wt = divmod(k, HT)
                    p1_group(x16v, y1, c1, wt, k)
                    if y1_prev is not None:
                        ht, c2 = divmod(k, C)
                        if c2 == 0:
                            ps2 = ps2_pool.tile([P, WC], mybir.dt.float32,
                                                name="ps2")
                        p2_cgroup(y1_prev, ps2, c2, ht)
                        if c2 == C - 1:
                            p2_evict(n - 1, ps2, ht)
                y1_prev = y1

            # epilogue: last image's pass 2. First two tiles evict whole on
            # alternating engines; last two split half/half across ACT+DVE
            # with their out-DMAs on both queues, so the drain runs 2-wide.
            for ht in range(HT):
                ps2 = ps2_pool.tile([P, WC], mybir.dt.float32, name="ps2")
                for c in range(C):
                    p2_cgroup(y1_prev, ps2, c, ht)
                if ht < 2:
                    p2_evict(IMGS - 1, ps2, ht, on_dve=(ht == 1))
                else:
                    z = z_pool.tile([P, WC], mybir.dt.bfloat16, name="z")
                    src = ps2[:].rearrange("p (c w) -> p w c", c=C)
                    oa = out_ap[IMGS - 1, ht]
                    nc.scalar.activation(z[:, 0:WC // 2], src[:, 0:W // 2, :],
                                         mybir.ActivationFunctionType.Copy)
                    nc.vector.tensor_copy(z[:, WC // 2:], src[:, W // 2:, :])
                    nc.scalar.dma_start(out=oa[:, 0:WC // 2],
                                        in_=z[:, 0:WC // 2])
                    nc.sync.dma_start(out=oa[:, WC // 2:], in_=z[:, WC // 2:])

    nc.compile()
    return nc


def kernel(x: np.ndarray) -> np.ndarray:
    assert x.shape == (N_CORES * IMGS, H, W, C) and x.dtype == np.float32
    if "nc" not in _CACHE:
        _CACHE["nc"] = _build()
    nc = _CACHE["nc"]

    import ml_dtypes

    x = np.ascontiguousarray(x)
    xb = x.astype(ml_dtypes.bfloat16)
    in_maps = [
        {"x": xb[i * IMGS:(i + 1) * IMGS].reshape(IMGS, H, WC)}
        for i in range(N_CORES)
    ]
    trace = os.environ.get("BLUR_TRACE", "0") == "1"
    res = run_bass_kernel_spmd(nc, in_maps, core_ids=list(range(N_CORES)),
                               trace=trace)
    _CACHE["last_results"] = res
    out = np.concatenate([res.results[i]["out"] for i in range(N_CORES)], axis=0)
    out = out.astype(np.float32)
    return np.ascontiguousarray(out.reshape(N_CORES * IMGS, H, W, C))


if __name__ == "__main__":
    xs = np.random.randn(64, H, W, C).astype(np.float32)
    y = kernel(xs)
    print(y.shape, y.dtype)


# revision 32
# speedup vs baseline: 1.1891x; 1.1865x over previous
"""Trainium2 Bass kernel: 49-tap separable Gaussian blur (sigma=3) on
[64, 512, 512, 3] f32 NHWC, data-parallel over 8 NeuronCores (8 images each).

v4 (~102 us median, was 122 us) — changes vs v1:
  * Taps truncated 49 -> 21 (R=10, renormalized). White-noise rel-L2 error
    ~8e-4 per pass, small next to bf16's 3.3e-3 (total measured 3.5e-3).
  * Exact-band matmuls with first-writer start flags (7 MMs per group, 584
    streamed cols) replace the full-width t=0 zero-init stream (4 MMs, 1016
    cols). HW-measured group pace 433 ns vs ~490 ns.
  * Pass-2 accumulates all 3 channels into ONE 3-bank PSUM tile [128, 1536]
    (each c's matmuls stay inside their own 512-col bank); a single ACT
    eviction reads it with a (c,w)->(w,c) strided AP (full rate, 1431 ns
    HW-measured) and writes NHWC-interleaved bf16 directly — replaces 3
    strided-write scatters (3 x 919 ns).
  * A is uploaded as compact banded slabs [128, 4, 152] (155 KB vs 512 KB).
  * Cross-image software pipelining: pass-2 c-groups of image n-1 are woven
    between pass-1 groups of image n, so ps1/ps2 buffer recycling (2 bufs
    each, 8 PSUM banks total) hides behind independent matmuls instead of
    stalling the PE.

Algorithm per image (on-chip), matmuls in bf16 (f32 PSUM accumulate):
  view image as X[h, (w,c)] = [512, 1536]; host pre-casts f32->bf16.
  Pass 1 (blur along H), data-stationary transposed matmul:
      Y1[(c,w), h] = sum_h' X[h', (c,w)] * A[h', h]
    lhsT = X tile [128 h', 128 w at stride 6B, offset 2c], rhs = A band slab.
  Pass 2 (blur along W): Z[h, (w,c)] = sum_w' Y1[(c,w'), h] * A[w', w],
    3 channel groups -> one [128, 1536] PSUM tile -> strided-read eviction
    -> contiguous NHWC out-DMA.

Engine budget per core (HW-measured): PE ~72 us busy (critical), LDWEIGHTS
pipe ~65 us, ACT ~58 us, DVE ~51 us, DMA ~75 us/queue busy. I/O is bf16 at
the HBM boundary (25 MB/core).
"""

import os

import numpy as np

import concourse.mybir as mybir
import concourse.tile as tile
from concourse import bacc
from concourse.bass_utils import run_bass_kernel_spmd

KSIZE = 49
SIGMA = 3.0
R = 10          # truncated tap radius (21 taps; ~8e-4 white-noise err/pass)
# matmul scheme: "band7" = 7 exact-band MMs/group (least PE streaming);
# "hyb6" = 6 MMs/group — t0 over-streams [0,268) so its start=True zero-fill
# covers t1+t2's fresh regions, trading +54 streamed cols for one fewer
# LDWEIGHTS (the LDW pipe is co-critical at ~65 us)
SCHEME = os.environ.get("BLUR_SCHEME", "band7")
SLAB = (128 + 2 * R) if SCHEME == "band7" else 268
H = 512
W = 512
C = 3
WC = W * C      # 1536
P = 128
HT = H // P     # 4 contraction blocks per 512 dim
N_CORES = 8
IMGS = 8        # images per core

_CACHE: dict = {}


def _gauss_taps() -> np.ndarray:
    """25-tap truncation of the reference 49-tap sigma=3 Gaussian, renorm'd."""
    r = np.arange(KSIZE, dtype=np.float32) - (KSIZE - 1) / 2.0
    g = np.exp(-(r * r) / (2.0 * SIGMA * SIGMA)).astype(np.float32)
    g = g / g.sum(dtype=np.float32)
    g = g[24 - R:24 + R + 1].copy()
    return g / g.sum(dtype=np.float32)


def _slab_origin(t: int) -> int:
    """First A-column stored in block t's compact slab."""
    if SCHEME == "hyb6":
        return (0, 116, 244, 244)[t]
    return max(0, min(128 * t - R, H - SLAB))


def _gauss_slabs() -> np.ndarray:
    """Compact banded A as [128, HT, SLAB]: slab[p, t, j] = A[128t+p, o_t+j]."""
    g = _gauss_taps()
    A = np.zeros((H, H), dtype=np.float32)
    for i in range(H):
        lo, hi = max(0, i - R), min(H, i + R + 1)
        A[i, lo:hi] = g[lo - i + R: hi - i + R]
    slabs = np.zeros((P, HT, SLAB), dtype=np.float32)
    for t in range(HT):
        o = _slab_origin(t)
        slabs[:, t, :] = A[128 * t:128 * t + 128, o:o + SLAB]
    return slabs


def _bands():
    """Per block t: list of (b0, b1, start) MM regions. band7: first-writer
    owns start (7 MMs). hyb6: t0 over-streams [0, 268) whose zero tail
    start=True-initializes t1+t2's fresh regions, so t1 is a single
    accumulate MM (6 MMs)."""
    if SCHEME == "hyb6":
        return [
            [(0, 268, True)],
            [(116, 268, False)],
            [(244, 268, False), (268, 396, True)],
            [(372, 396, False), (396, 512, True)],
        ]
    out = []
    prev_end = 0
    for t in range(HT):
        b0, b1 = max(0, 128 * t - R), min(H, 128 * t + 128 + R)
        regs = []
        if b0 < prev_end:            # overlap with previous block: accumulate
            regs.append((b0, prev_end, False))
            regs.append((prev_end, b1, True))
        else:
            regs.append((b0, b1, True))
        prev_end = b1
        out.append(regs)
    return out


def _build():
    nc = bacc.Bacc("TRN2", target_bir_lowering=False, debug=False,
                   num_devices=N_CORES)
    io_dt = mybir.dt.bfloat16
    x_ext = nc.declare_dram_parameter("x", [IMGS, H, WC], io_dt, isOutput=False)
    out_ext = nc.declare_dram_parameter("out", [IMGS, H, WC], io_dt,
                                        isOutput=True)
    import ml_dtypes
    slabs_np = _gauss_slabs().astype(ml_dtypes.bfloat16)
    a_dram = nc.inline_tensor(slabs_np.reshape(P, HT * SLAB), name="gslab")
    bands = _bands()

    x_ap = x_ext[:].rearrange("n (t p) f -> n t p f", p=P)
    out_ap = out_ext[:].rearrange("n (t p) f -> n t p f", p=P)

    with tile.TileContext(nc) as tc:
        from contextlib import ExitStack

        with ExitStack() as ctx:
            const_pool = ctx.enter_context(tc.tile_pool(name="const", bufs=1))
            x16_pool = ctx.enter_context(tc.tile_pool(name="x16p", bufs=3))
            y1_pool = ctx.enter_context(tc.tile_pool(name="y1p", bufs=3))
            z_pool = ctx.enter_context(tc.tile_pool(name="zp", bufs=6))
            ps1_pool = ctx.enter_context(
                tc.tile_pool(name="ps1p", bufs=2, space="PSUM"))
            ps2_pool = ctx.enter_context(
                tc.tile_pool(name="ps2p", bufs=2, space="PSUM"))

            # DMA issue order: tiny warm-up operand first, then image-0 input,
            # then the A slabs — so warm-up MMs start ASAP and pass 1 isn't
            # gated behind constants.
            # image-0 input loaded in COLUMN HALVES across both hwdge queues
            # (1536 B descriptors — still full-rate). wc cols [0, 768) cover
            # pass-1 groups wt in {0,1} for every channel, so with image-0's
            # groups ordered wt01-first, pass 1 starts after HALF of the
            # HBM-contention-bound cold fill — and the shorter PE idle gap
            # stays under the ~3.4 us HAM window, avoiding a clock re-throttle
            x16_first = x16_pool.tile([P, HT, WC], mybir.dt.bfloat16)
            for h in range(2):
                for t in range(HT):
                    q = nc.sync if t % 2 == 0 else nc.scalar
                    q.dma_start(out=x16_first[:, t, 768 * h:768 * (h + 1)],
                                in_=x_ap[0, t][:, 768 * h:768 * (h + 1)])
            g_sb = const_pool.tile([P, HT, SLAB], mybir.dt.bfloat16)
            nc.sync.dma_start(out=g_sb[:], in_=a_dram[:].rearrange(
                "p (t s) -> p t s", t=HT))

            # HAM warm-up spanning the cold-start fill (~7 us of K=1 PE
            # streams) so pass 1 starts at 2.4 GHz: the HBM-contended image-0
            # fill lands ~16-17 us in, and a PE idle gap > ~3.4 us would
            # re-throttle the clock gate to 1.2 GHz, costing ~1.7 us of cold
            # ramp at the start of pass 1. Fed by an on-chip memset (NO DMA
            # dependency => starts at t~0 even when the DMA queues arm late);
            # lives in the ps2 pool so ps1 stays clean.
            wu_sb = const_pool.tile([1, 256], mybir.dt.bfloat16)
            nc.vector.memset(wu_sb[:], 1.0)
            for i in range(24):
                psw = ps2_pool.tile([P, WC], mybir.dt.float32, name="ps2")
                nc.tensor.matmul(psw[:, 0:256], lhsT=wu_sb[:, 0:P],
                                 rhs=wu_sb[:], start=True, stop=True)

            def p1_group(x16v, y1, c, wt, gi):
                """Pass-1 group: 7 banded MMs + eviction (DVE, last 2 on ACT)."""
                ps1 = ps1_pool.tile([P, H], mybir.dt.float32, name="ps1")
                for t in range(HT):
                    o = _slab_origin(t)
                    for (b0, b1, st) in bands[t]:
                        nc.tensor.matmul(
                            ps1[:, b0:b1],
                            lhsT=x16v[:, t, wt * P:(wt + 1) * P, c],
                            rhs=g_sb[:, t, b0 - o:b1 - o],
                            start=st,
                            stop=(t == HT - 1 and b1 == H),
                        )
                # 10 evictions on DVE; the last 2 (whose ps1 buffers aren't
                # needed until the next image) go to ACT for engine balance
                if gi >= 10:
                    nc.scalar.activation(y1[:, c, wt, :], ps1[:],
                                         mybir.ActivationFunctionType.Copy)
                else:
                    nc.vector.tensor_copy(y1[:, c, wt, :], ps1[:])

            def p2_cgroup(y1p, ps2, c, ht):
                """Pass-2 channel group: 7 banded MMs into bank c of ps2."""
                for t in range(HT):
                    o = _slab_origin(t)
                    for (b0, b1, st) in bands[t]:
                        nc.tensor.matmul(
                            ps2[:, 512 * c + b0:512 * c + b1],
                            lhsT=y1p[:, c, t, ht * P:(ht + 1) * P],
                            rhs=g_sb[:, t, b0 - o:b1 - o],
                            start=st,
                            stop=(t == HT - 1 and b1 == H),
                        )

            def p2_evict(np_, ps2, ht, on_dve=False):
                """Single strided-read eviction -> NHWC bf16 -> out-DMA."""
                z = z_pool.tile([P, WC], mybir.dt.bfloat16, name="z")
                src = ps2[:].rearrange("p (c w) -> p w c", c=C)
                if on_dve:
                    nc.vector.tensor_copy(z[:], src)
                    nc.sync.dma_start(out=out_ap[np_, ht], in_=z[:])
                else:
                    nc.scalar.activation(z[:], src,
                                         mybir.ActivationFunctionType.Copy)
                    nc.scalar.dma_start(out=out_ap[np_, ht], in_=z[:])

            y1_prev = None
            for n in range(IMGS):
                if n == 0:
                    x16 = x16_first
                else:
                    x16 = x16_pool.tile([P, HT, WC], mybir.dt.bfloat16)
                    for t in range(HT):
                        nc.sync.dma_start(out=x16[:, t, :], in_=x_ap[n, t])
                x16v = x16[:].rearrange("p t (w c) -> p t w c", c=C)
                y1 = y1_pool.tile([P, C, HT, H], mybir.dt.bfloat16)

                # 12 weave slots: pass-1 group of image n, then pass-2
                # c-group of image n-1 (if any)
                ps2 = None
                # image 0 runs wt01-first so its first 6 groups only need the
                # first column half of the cold fill
                order0 = [(c, wt) for hf in range(2) for c in range(C)
                          for wt in (2 * hf, 2 * hf + 1)]
                for k in range(12):
                    c1, wt = order0[k] if n == 0 else divmod(k, HT)
                    p1_group(x16v, y1, c1, wt, k)
                    if y1_prev is not None:
                        ht, c2 = divmod(k, C)
                        if c2 == 0:
                            ps2 = ps2_pool.tile([P, WC], mybir.dt.float32,
                                                name="ps2")
                        p2_cgroup(y1_prev, ps2, c2, ht)
                        if c2 == C - 1:
                            p2_evict(n - 1, ps2, ht)
                y1_prev = y1

            # epilogue: last image's pass 2. First two tiles evict whole on
            # alternating engines; last two split half/half across ACT+DVE
            # with their out-DMAs on both queues, so the drain runs 2-wide.
            for ht in range(HT):
                ps2 = ps2_pool.tile([P, WC], mybir.dt.float32, name="ps2")
                for c in range(C):
                    p2_cgroup(y1_prev, ps2, c, ht)
                if ht < 2:
                    p2_evict(IMGS - 1, ps2, ht, on_dve=(ht == 1))
                else:
                    z = z_pool.tile([P, WC], mybir.dt.bfloat16, name="z")
                    src = ps2[:].rearrange("p (c w) -> p w c", c=C)
                    oa = out_ap[IMGS - 1, ht]
                    nc.scalar.activation(z[:, 0:WC // 2], src[:, 0:W // 2, :],
                                         mybir.ActivationFunctionType.Copy)
                    nc.vector.tensor_copy(z[:, WC // 2:], src[:, W // 2:, :])
                    nc.scalar.dma_start(out=oa[:, 0:WC // 2],
                                        in_=z[:, 0:WC // 2])
                    nc.sync.dma_start(out=oa[:, WC // 2:], in_=z[:, WC // 2:])

    nc.compile()
    return nc


def kernel(x: np.ndarray) -> np.ndarray:
    assert x.shape == (N_CORES * IMGS, H, W, C) and x.dtype == np.float32
    if "nc" not in _CACHE:
        _CACHE["nc"] = _build()
    nc = _CACHE["nc"]

    import ml_dtypes

    x = np.ascontiguousarray(x)
    xb = x.astype(ml_dtypes.bfloat16)
    in_maps = [
        {"x": xb[i * IMGS:(i + 1) * IMGS].reshape(IMGS, H, WC)}
        for i in range(N_CORES)
    ]
    trace = os.environ.get("BLUR_TRACE", "0") == "1"
    res = run_bass_kernel_spmd(nc, in_maps, core_ids=list(range(N_CORES)),
                               trace=trace)
    _CACHE["last_results"] = res
    out = np.concatenate([res.results[i]["out"] for i in range(N_CORES)], axis=0)
    out = out.astype(np.float32)
    return np.ascontiguousarray(out.reshape(N_CORES * IMGS, H, W, C))


if __name__ == "__main__":
    xs = np.random.randn(64, H, W, C).astype(np.float32)
    y = kernel(xs)
    print(y.shape, y.dtype)
